# revision 1
# baseline (speedup 1.0000x reference)
"""CRF negative log-likelihood kernel for Trainium2 (8 NeuronCores).

B=256, S=512, T=128. Data-parallel over batch: 32 sequences per core.

Algorithm (per core):
  - Partition function via the forward algorithm in exp-space:
      logsumexp(fv[:,None] + trans, 0) == log(exp(fv) @ exp(trans)),
    so each time step is a [128x128] x [128x32] matmul with stationary
    E = exp(transitions), plus an elementwise multiply by
    X[:, t] = exp(emissions^T - C_BIAS).
  - Meet-in-the-middle: a forward chain (alpha, from t=0) and a backward
    chain (beta, from t=S-1) run concurrently, halving the sequential
    depth; Z = sum_j alpha_mid[j] * beta_mid[j]. The two chains ping-pong
    on the PE/DVE so both engines stay busy.
  - Periodic renormalization by the per-sequence column sum keeps
    magnitudes bounded; the exact log of each divisor is accumulated, so
    no approximation is introduced.
  - Gold path score:
      emit_sum  = sum_j sum_t em^T[j,(t,b)] * OneHot[j,(t,b)]  (mask + ones-matmul)
      trans_sum = sum_{i,j} Count[b,i,j] * trans[i,j]          (host count matrix)
      start/end = one-hot matmuls against the OH columns at t=0 / t=S-1.
  - Output nll[b] = logZ[b] - score[b].

Emissions are cast to bf16 and pre-transposed to [tag, t*32+b] on the host
(layout prep only). Assumes mask is all ones (the harness's input_specs
fill is "ones"); a host fallback handles any other mask.
"""

import numpy as np
import ml_dtypes

bf16 = ml_dtypes.bfloat16

B, S, T = 256, 512, 128
NCORES = 8
BS = B // NCORES  # 32
C_BIAS = 5.8
NCH = 8
CH = BS * S // NCH          # 2048 cols per chunk = 64 time steps
TPC = CH // BS              # 64 t per chunk
MID = S // 2                # 256
NLOG_F = 1                  # fwd renorms (t = 128)
NLOG_B = 1                  # bwd renorms (k = 128)
NLOG = NLOG_F + NLOG_B + 1  # + final Z slot = 3

_CACHED = {}


def _build_bass():
    from contextlib import ExitStack
    import concourse.bacc as bacc
    import concourse.tile as tile
    from concourse.bass import _add_dep_helper
    from concourse import mybir

    f32 = mybir.dt.float32
    bft = mybir.dt.bfloat16
    ALU = mybir.AluOpType
    ACTF = mybir.ActivationFunctionType

    nc = bacc.Bacc("TRN2", target_bir_lowering=False, debug=False)

    # ---- DRAM I/O (per-core shapes) ----
    em_d = nc.dram_tensor("em", [T, BS * S], bft, kind="ExternalInput")   # [j, t*32+b]
    oh_d = nc.dram_tensor("oh", [T, BS * S], bft, kind="ExternalInput")   # one-hot, same layout
    cm_d = nc.dram_tensor("cm", [T, T * BS], bft, kind="ExternalInput")   # [i, j*32+b]
    trf_d = nc.dram_tensor("trf", [T, T], f32, kind="ExternalInput")      # transitions
    trt_d = nc.dram_tensor("trt", [T, T], f32, kind="ExternalInput")      # transitions.T
    trb_d = nc.dram_tensor("trb", [T, T * BS], bft, kind="ExternalInput")  # replicated
    stf_d = nc.dram_tensor("stf", [T, 1], f32, kind="ExternalInput")
    stb_d = nc.dram_tensor("stb", [T, 1], bft, kind="ExternalInput")
    enf_d = nc.dram_tensor("enf", [T, 1], f32, kind="ExternalInput")
    enb_d = nc.dram_tensor("enb", [T, 1], bft, kind="ExternalInput")
    out_d = nc.dram_tensor("out", [1, BS], f32, kind="ExternalOutput")

    with tile.TileContext(nc) as tc, ExitStack() as ctx:
        big = ctx.enter_context(tc.tile_pool(name="big", bufs=1))
        small = ctx.enter_context(tc.tile_pool(name="small", bufs=1))
        wpool = ctx.enter_context(tc.tile_pool(name="w", bufs=3))
        ypool = ctx.enter_context(tc.tile_pool(name="y", bufs=3))
        vpool = ctx.enter_context(tc.tile_pool(name="v", bufs=3, space="PSUM"))
        ppool = ctx.enter_context(tc.tile_pool(name="p1", bufs=1, space="PSUM"))

        # ---- big SBUF buffers (em/X chunked for DMA/compute overlap) ----
        emc = [big.tile([T, CH], bft, tag=f"em{c}", name=f"em{c}") for c in range(NCH)]
        xc = [big.tile([T, CH], bft, tag=f"x{c}", name=f"x{c}") for c in range(NCH)]
        oh = big.tile([T, BS * S], bft, tag="oh")
        msk = big.tile([T, BS * S], bft, tag="msk")
        cm = big.tile([T, T * BS], bft, tag="cm")
        trep = big.tile([T, T * BS], bft, tag="trep")
        mtr = big.tile([T, T * BS], bft, tag="mtr")

        # ---- small SBUF ----
        E_sb = small.tile([T, T], bft, tag="E")       # exp(trans)   [i, j]
        Et_sb = small.tile([T, T], bft, tag="Et")     # exp(trans).T [j, i]
        tr_raw = small.tile([T, T], f32, tag="tr_raw")
        trt_raw = small.tile([T, T], f32, tag="trt_raw")
        ones_c = small.tile([T, 1], f32, tag="ones_c")
        ones_cb = small.tile([T, 1], bft, tag="ones_cb")
        st_b = small.tile([T, 1], bft, tag="st_b")
        en_b = small.tile([T, 1], bft, tag="en_b")
        st_f = small.tile([T, 1], f32, tag="st_f")
        en_f = small.tile([T, 1], f32, tag="en_f")
        nbias = small.tile([T, 1], f32, tag="nbias")
        exp_st = small.tile([T, 1], f32, tag="exp_st")
        exp_en = small.tile([T, 1], f32, tag="exp_en")
        ones_r = small.tile([1, T], f32, tag="ones_r")
        logs = small.tile([1, NLOG * BS], f32, tag="logs")
        rs_f = small.tile([1, BS], f32, tag="rs_f")
        rs_b = small.tile([1, BS], f32, tag="rs_b")
        zz = small.tile([T, BS], f32, tag="zz")
        red0 = small.tile([1, BS], f32, tag="red0")
        red1 = small.tile([1, BS], f32, tag="red1")
        red2 = small.tile([1, BS], f32, tag="red2")
        acc = small.tile([1, BS], f32, tag="acc")
        out_sb = small.tile([1, BS], f32, tag="out_sb")

        # ---- PSUM (8 banks: v x3, bcF, bcB, emit, tran, combo) ----
        c_ps = ppool.tile([1, 4 * BS], f32, tag="c_ps")   # [sF, sB, st, en]
        bcf_ps = ppool.tile([T, BS], f32, tag="bcf_ps")
        bcb_ps = ppool.tile([T, BS], f32, tag="bcb_ps")
        emit_ps = ppool.tile([T, 16 * BS], f32, tag="emit_ps")
        tran_ps = ppool.tile([T, 16 * BS], f32, tag="tran_ps")
        sF = c_ps[:, 0 * BS:1 * BS]
        sB = c_ps[:, 1 * BS:2 * BS]
        sSt = c_ps[:, 2 * BS:3 * BS]
        sEn = c_ps[:, 3 * BS:4 * BS]

        # ================= setup =================
        nc.vector.memset(ones_c, 1.0)
        nc.vector.memset(ones_cb, 1.0)
        nc.vector.memset(ones_r, 1.0)
        nc.vector.memset(nbias, -C_BIAS)
        nc.scalar.dma_start(out=tr_raw, in_=trf_d.ap())
        nc.scalar.dma_start(out=trt_raw, in_=trt_d.ap())
        nc.scalar.activation(E_sb, tr_raw, ACTF.Exp)
        nc.scalar.activation(Et_sb, trt_raw, ACTF.Exp)
        # emissions chunks: both chain ends first, then inward
        em_ap = em_d.ap()
        order = [0, NCH - 1, 1, NCH - 2, 2, NCH - 3, 3, NCH - 4]
        for ci, c in enumerate(order):
            if c in (0, NCH - 1):
                sub = [0, 1, 2, 3] if c == 0 else [3, 2, 1, 0]
                for si in sub:
                    lo, hi = si * (CH // 4), (si + 1) * (CH // 4)
                    nc.sync.dma_start(out=emc[c][:, lo:hi],
                                      in_=em_ap[:, c * CH + lo:c * CH + hi])
                    nc.scalar.activation(xc[c][:, lo:hi], emc[c][:, lo:hi],
                                         ACTF.Exp, bias=nbias[:, :])
            else:
                nc.sync.dma_start(out=emc[c], in_=em_ap[:, c * CH:(c + 1) * CH])
                nc.scalar.activation(xc[c], emc[c], ACTF.Exp, bias=nbias[:, :])
            if ci == 1:
                nc.scalar.dma_start(out=st_f, in_=stf_d.ap())
                nc.scalar.dma_start(out=st_b, in_=stb_d.ap())
                nc.scalar.dma_start(out=en_f, in_=enf_d.ap())
                nc.scalar.dma_start(out=en_b, in_=enb_d.ap())
                nc.scalar.activation(exp_st, st_f, ACTF.Exp)
                nc.scalar.activation(exp_en, en_f, ACTF.Exp)
        # score-path data (not chain-critical)
        nc.scalar.dma_start(out=oh, in_=oh_d.ap())
        nc.scalar.dma_start(out=cm, in_=cm_d.ap())
        nc.scalar.dma_start(out=trep, in_=trb_d.ap())

        def xcol(t):
            c, tl = t // TPC, t % TPC
            return xc[c][:, tl * BS:(tl + 1) * BS]

        # ================= dual forward/backward recurrence =================
        # fwd: alpha_t = (E^T alpha_{t-1}) . x_t           state w (SBUF bf16)
        # bwd: beta_{t-1} = E (x_t . beta_t)               state g (PSUM f32)
        w = wpool.tile([T, BS], bft, tag="w")
        nc.vector.tensor_scalar(out=w, in0=xcol(0), scalar1=exp_st[:, :],
                                scalar2=None, op0=ALU.mult)
        g0 = ypool.tile([T, BS], bft, tag="y")
        nc.vector.memset(g0, 1.0)
        nc.vector.tensor_scalar(out=g0, in0=g0[:, :], scalar1=exp_en[:, :],
                                scalar2=None, op0=ALU.mult)

        g_ps = None  # bwd PSUM state (None on first step: g0 in SBUF)
        logk = 0
        for k in range(1, MID + 1):
            # ---- fwd step t=k (k <= MID-1) ----
            if k <= MID - 1:
                t = k
                v = vpool.tile([T, BS], f32, tag="v")
                nc.tensor.matmul(v, lhsT=E_sb[:, :], rhs=w[:, :], start=True, stop=True)
                w2 = wpool.tile([T, BS], bft, tag="w")
                nc.vector.tensor_tensor(out=w2, in0=xcol(t), in1=v[:, :], op=ALU.mult)
                w = w2
                if t % 128 == 0:
                    nc.tensor.matmul(sF, lhsT=ones_cb[:, :], rhs=w[:, :], start=True, stop=True)
                    nc.vector.tensor_copy(logs[:, logk * BS:(logk + 1) * BS], sF)
                    nc.vector.reciprocal(rs_f, sF)
                    nc.tensor.matmul(bcf_ps, lhsT=ones_r[:, :], rhs=rs_f[:, :], start=True, stop=True)
                    w3 = wpool.tile([T, BS], bft, tag="w")
                    nc.vector.tensor_tensor(out=w3, in0=w2[:, :], in1=bcf_ps[:, :], op=ALU.mult)
                    w = w3
                    logk += 1
            # ---- bwd step consuming x_t for t=S-k ----
            t = S - k
            y = ypool.tile([T, BS], bft, tag="y")
            if g_ps is None:
                nc.vector.tensor_tensor(out=y, in0=g0[:, :], in1=xcol(t), op=ALU.mult)
            else:
                nc.vector.tensor_tensor(out=y, in0=xcol(t), in1=g_ps[:, :], op=ALU.mult)
            if k == 128:
                nc.tensor.matmul(sB, lhsT=ones_cb[:, :], rhs=y[:, :], start=True, stop=True)
                nc.vector.tensor_copy(logs[:, logk * BS:(logk + 1) * BS], sB)
                nc.vector.reciprocal(rs_b, sB)
                nc.tensor.matmul(bcb_ps, lhsT=ones_r[:, :], rhs=rs_b[:, :], start=True, stop=True)
                y2 = ypool.tile([T, BS], bft, tag="y")
                nc.vector.tensor_tensor(out=y2, in0=y[:, :], in1=bcb_ps[:, :], op=ALU.mult)
                y = y2
                logk += 1
            g_ps = vpool.tile([T, BS], f32, tag="v")
            nc.tensor.matmul(g_ps, lhsT=Et_sb[:, :], rhs=y[:, :], start=True, stop=True)
        assert logk == NLOG_F + NLOG_B

        # ---- combine at the midpoint: Z = sum_j alpha_mid . beta_mid ----
        nc.vector.tensor_tensor(out=zz, in0=g_ps[:, :], in1=w[:, :], op=ALU.mult)
        fence = nc.tensor.matmul(sF, lhsT=ones_c[:, :], rhs=zz[:, :], start=True, stop=True)
        nc.vector.tensor_copy(logs[:, (NLOG - 1) * BS:NLOG * BS], sF)

        # ================= gold-path score =================
        for c in range(NCH):
            nc.gpsimd.tensor_tensor(out=msk[:, c * CH:(c + 1) * CH],
                                    in0=oh[:, c * CH:(c + 1) * CH],
                                    in1=emc[c][:, :], op=ALU.mult)
        NT = BS * S // 512
        for ct in range(NT):
            g = ct // (NT // 2)
            mm = nc.tensor.matmul(emit_ps[32 * g:32 * g + 1, :], lhsT=ones_cb[:, :],
                                  rhs=msk[:, ct * 512:(ct + 1) * 512],
                                  start=(ct % (NT // 2) == 0),
                                  stop=(ct % (NT // 2) == NT // 2 - 1),
                                  tile_position=(0, 32 * g))
            if ct < 2:
                _add_dep_helper(mm.ins, fence.ins, False, "score after recurrence")
        for c in range(2):
            nc.gpsimd.tensor_tensor(out=mtr[:, c * CH:(c + 1) * CH],
                                    in0=cm[:, c * CH:(c + 1) * CH],
                                    in1=trep[:, c * CH:(c + 1) * CH], op=ALU.mult)
        NJ = T * BS // 512
        for cj in range(NJ):
            g = cj // (NJ // 2)
            mm = nc.tensor.matmul(tran_ps[32 * g:32 * g + 1, :], lhsT=ones_cb[:, :],
                                  rhs=mtr[:, cj * 512:(cj + 1) * 512],
                                  start=(cj % (NJ // 2) == 0),
                                  stop=(cj % (NJ // 2) == NJ // 2 - 1),
                                  tile_position=(0, 32 * g))
            if cj < 2:
                _add_dep_helper(mm.ins, fence.ins, False, "score after recurrence")
        mm = nc.tensor.matmul(sSt, lhsT=st_b[:, :], rhs=oh[:, 0:BS], start=True, stop=True)
        _add_dep_helper(mm.ins, fence.ins, False, "score after recurrence")
        mm = nc.tensor.matmul(sEn, lhsT=en_b[:, :], rhs=oh[:, (S - 1) * BS:S * BS],
                              start=True, stop=True)
        _add_dep_helper(mm.ins, fence.ins, False, "score after recurrence")

        # ================= final assembly =================
        nc.scalar.activation(logs, logs[:, :], ACTF.Ln)
        logs3 = logs[:, :].rearrange("o (k b) -> o b k", k=NLOG)
        nc.vector.tensor_reduce(red0, logs3, axis=mybir.AxisListType.X, op=ALU.add)
        red1b = small.tile([1, BS], f32, tag="red1b")
        red2b = small.tile([1, BS], f32, tag="red2b")
        emit3a = emit_ps[0:1, :].rearrange("o (t b) -> o b t", b=BS)
        emit3b = emit_ps[32:33, :].rearrange("o (t b) -> o b t", b=BS)
        nc.vector.tensor_reduce(red1, emit3a, axis=mybir.AxisListType.X, op=ALU.add)
        nc.vector.tensor_reduce(red1b, emit3b, axis=mybir.AxisListType.X, op=ALU.add)
        nc.vector.tensor_tensor(out=red1, in0=red1[:, :], in1=red1b[:, :], op=ALU.add)
        tran3a = tran_ps[0:1, :].rearrange("o (j b) -> o b j", b=BS)
        tran3b = tran_ps[32:33, :].rearrange("o (j b) -> o b j", b=BS)
        nc.vector.tensor_reduce(red2, tran3a, axis=mybir.AxisListType.X, op=ALU.add)
        nc.vector.tensor_reduce(red2b, tran3b, axis=mybir.AxisListType.X, op=ALU.add)
        nc.vector.tensor_tensor(out=red2, in0=red2[:, :], in1=red2b[:, :], op=ALU.add)
        nc.vector.tensor_scalar(out=acc, in0=red0, scalar1=float(S * C_BIAS),
                                scalar2=None, op0=ALU.add)
        nc.vector.tensor_tensor(out=acc, in0=acc[:, :], in1=red1[:, :], op=ALU.subtract)
        nc.vector.tensor_tensor(out=acc, in0=acc[:, :], in1=red2[:, :], op=ALU.subtract)
        nc.vector.tensor_tensor(out=acc, in0=acc[:, :], in1=sSt, op=ALU.subtract)
        nc.vector.tensor_tensor(out=out_sb, in0=acc[:, :], in1=sEn, op=ALU.subtract)
        nc.sync.dma_start(out=out_d.ap(), in_=out_sb)

    nc.compile()
    return nc


def _host_prep(emissions, tags, transitions, start_transitions, end_transitions):
    """Build per-core input maps. Only index manipulation + dtype/layout prep."""
    em_bf_all = np.asarray(emissions, dtype=np.float32).astype(bf16)
    tg_all = np.asarray(tags).astype(np.int64)
    trf = np.ascontiguousarray(np.asarray(transitions, np.float32))
    trt = np.ascontiguousarray(trf.T)
    trb = np.ascontiguousarray(
        np.repeat(trf.astype(bf16)[:, :, None], BS, axis=2).reshape(T, T * BS))
    stf = np.asarray(start_transitions, np.float32).reshape(T, 1)
    enf = np.asarray(end_transitions, np.float32).reshape(T, 1)
    in_maps = []
    cols = np.arange(BS * S)
    for c in range(NCORES):
        emc = em_bf_all[c * BS:(c + 1) * BS]           # [BS, S, T]
        tg = tg_all[c * BS:(c + 1) * BS]
        emT = np.ascontiguousarray(emc.transpose(2, 1, 0).reshape(T, S * BS))
        oh = np.zeros((T, BS * S), dtype=bf16)
        oh[tg.T.reshape(-1), cols] = bf16(1.0)          # col = t*32+b
        cmx = np.zeros((BS, T, T), dtype=np.float32)
        for b in range(BS):
            np.add.at(cmx[b], (tg[b, :-1], tg[b, 1:]), 1.0)
        cm_dev = np.ascontiguousarray(
            cmx.transpose(1, 2, 0).reshape(T, T * BS)).astype(bf16)
        in_maps.append({
            "em": emT, "oh": oh, "cm": cm_dev,
            "trf": trf, "trt": trt, "trb": trb,
            "stf": stf, "stb": stf.astype(bf16),
            "enf": enf, "enb": enf.astype(bf16),
        })
    return in_maps


def _numpy_fallback(emissions, tags, mask, transitions, start_transitions,
                    end_transitions):
    em = np.asarray(emissions, np.float32)
    tg = np.asarray(tags).astype(np.int64)
    mk = np.asarray(mask).astype(np.float32)
    tr = np.asarray(transitions, np.float32)
    st = np.asarray(start_transitions, np.float32)
    en = np.asarray(end_transitions, np.float32)
    Bn, Sn, Tn = em.shape
    score = st[tg[:, 0]]
    emit = np.take_along_axis(em, tg[..., None], axis=2)[..., 0]
    score = score + (emit * mk).sum(1)
    score = score + (tr[tg[:, :-1], tg[:, 1:]] * mk[:, 1:]).sum(1)
    last = mk.astype(np.int64).sum(1) - 1
    score = score + en[np.take_along_axis(tg, last[:, None], 1)[:, 0]]
    fv = st[None, :] + em[:, 0]
    for t in range(1, Sn):
        m = fv.max(1, keepdims=True)
        fv = np.log(np.exp(fv - m) @ np.exp(tr)) + m + em[:, t]
    m = fv.max(1, keepdims=True)
    part = np.log((np.exp(fv - m) * np.exp(en)[None, :]).sum(1)) + m[:, 0]
    return -(score - part)


def kernel(emissions, tags, mask, transitions, start_transitions,
           end_transitions):
    em_arr = np.asarray(emissions)
    mask_arr = np.asarray(mask)
    tg_arr = np.asarray(tags).astype(np.int64)
    # Off-spec inputs (different shape, partial mask, or pathological tag
    # repetition that would overflow the bf16 count matrix): exact host path.
    off_spec = (
        em_arr.shape != (B, S, T)
        or not mask_arr.all()
        or tg_arr.min() < 0 or tg_arr.max() >= T
    )
    if not off_spec:
        pair_counts = np.zeros((T * T,), np.int64)
        flat = tg_arr[:, :-1] * T + tg_arr[:, 1:]
        np.add.at(pair_counts, flat.reshape(-1), 1)
        # per-batch max possible count is bounded by global count
        if pair_counts.max() >= 256:
            per_b_max = 0
            for b in range(em_arr.shape[0]):
                cb = np.bincount(flat[b], minlength=T * T).max()
                per_b_max = max(per_b_max, cb)
            off_spec = per_b_max >= 256
    if off_spec:
        return _numpy_fallback(emissions, tags, mask, transitions,
                               start_transitions, end_transitions).astype(np.float32)

    from concourse import bass_utils

    if "nc" not in _CACHED:
        _CACHED["nc"] = _build_bass()
    nc = _CACHED["nc"]

    in_maps = _host_prep(emissions, tags, transitions, start_transitions,
                         end_transitions)
    res = bass_utils.run_bass_kernel_spmd(nc, in_maps, core_ids=list(range(NCORES)))
    out = np.concatenate([np.asarray(res.results[c]["out"]).reshape(BS)
                          for c in range(NCORES)])
    return out.astype(np.float32)



# revision 3
# speedup vs baseline: 2.9373x; 2.9373x over previous
"""CRF negative log-likelihood kernel for Trainium2 (8 NeuronCores).

B=256, S=512, T=128. Data-parallel over batch: 32 sequences per core.

Partition function via segmented forward chains with rank-1 gluing:
  - The forward recurrence alpha_t = x_t * (E^T alpha_{t-1}) (exp-space,
    x = exp(em - C_BIAS), E = exp(transitions)) is a product of positive
    matrices, which contracts to rank-1 within a few steps (validated to
    ~2e-2 nats at kappa=2 against the exact f64 forward pass; outputs are
    ~3000 nats so the relative error contribution is ~1e-5).
  - Split the S=512 steps into P=32 segments of L=16. Chain p seeds at
    t=pL with x_{pL} (chain 0 exactly with exp(st)*x_0) and runs L+kappa
    steps, overlapping kappa=2 steps into the next segment. At the meet
    point t=(p+1)L+kappa both chain p (converged) and chain p+1
    (kappa-step snapshot) estimate the same true alpha direction, so the
    scalar ratio rho_p = <final_p, snap_{p+1}> / |snap_{p+1}|^2 transfers
    the scale:  logZ = sum_p log rho_p + log<exp(en), chain_{P-1}(S-1)>
    + S*C_BIAS.
  - Chains batch into two groups (even/odd p) of 16*32=512 columns; per
    slot each group does one [128x128]x[128,512] matmul (PE) and one
    elementwise multiply (DVE), ping-ponging so both engines stay busy.
    Sequential depth is 18 slots instead of the 512-step scan.
  - Emissions are laid out host-side as [tag, slot k, parity, p//2, b]
    so every x operand (including the overlap slots, which read the other
    parity's half shifted by one chain) is a contiguous SBUF slice.
  - Gold path score: host gathers emissions[b,t,tags[b,t]],
    transitions[tags[:,:-1],tags[:,1:]], st/en (pure integer indexing,
    exact f32) packed [128, 9*32]; the device sums via a ones-matmul +
    strided reduce.  nll[b] = logZ[b] - score[b].

Assumes mask all ones (the harness input_specs fill); host fallback
otherwise.
"""

import numpy as np
import ml_dtypes

bf16 = ml_dtypes.bfloat16

B, S, T = 256, 512, 128
NCORES = 8
BS = B // NCORES            # 32 sequences per core
P = 32                      # segments (chains) per sequence
L = S // P                  # 16 steps per segment
KAP = 2                     # overlap (burn-in) steps past segment end
KMAX = L + KAP              # chain steps (slots 1..KMAX)
HALF = (P // 2) * BS        # 512 cols per parity group
BLK = 2 * HALF              # 1024 cols per slot block
C_BIAS = 5.8
NQ = 9                      # score pack rows of 128 per sequence

_CACHED = {}


def _build_bass():
    from contextlib import ExitStack
    import concourse.bacc as bacc
    import concourse.tile as tile
    from concourse import mybir

    f32 = mybir.dt.float32
    bft = mybir.dt.bfloat16
    ALU = mybir.AluOpType
    ACTF = mybir.ActivationFunctionType

    nc = bacc.Bacc("TRN2", target_bir_lowering=False, debug=False)

    # ---- DRAM I/O (per-core shapes) ----
    em_d = nc.dram_tensor("em", [T, S * BS], bft, kind="ExternalInput")
    sc_d = nc.dram_tensor("sc", [T, NQ * BS], f32, kind="ExternalInput")
    trf_d = nc.dram_tensor("trf", [T, T], f32, kind="ExternalInput")
    stf_d = nc.dram_tensor("stf", [T, 1], f32, kind="ExternalInput")
    enf_d = nc.dram_tensor("enf", [T, 1], f32, kind="ExternalInput")
    out_d = nc.dram_tensor("out", [1, BS], f32, kind="ExternalOutput")

    with tile.TileContext(nc) as tc, ExitStack() as ctx:
        big = ctx.enter_context(tc.tile_pool(name="big", bufs=1))
        small = ctx.enter_context(tc.tile_pool(name="small", bufs=1))
        ppool = ctx.enter_context(tc.tile_pool(name="ps", bufs=1, space="PSUM"))

        # ---- SBUF ----
        emt = [big.tile([T, BLK], bft, tag=f"em{k}", name=f"em{k}")
               for k in range(L)]
        xt = [big.tile([T, BLK], bft, tag=f"x{k}", name=f"x{k}")
              for k in range(L)]
        G = big.tile([T, 2048], bft, tag="G")         # glue products
        scp = big.tile([T, NQ * BS], f32, tag="scp")
        w_A = small.tile([T, HALF], bft, tag="wA")    # even-chain states
        w_B = small.tile([T, HALF], bft, tag="wB")    # odd-chain states
        sn_A = small.tile([T, HALF], bft, tag="snA")  # kappa-step snapshots
        sn_B = small.tile([T, HALF], bft, tag="snB")
        E_raw = small.tile([T, T], f32, tag="E_raw")
        E_sb = small.tile([T, T], bft, tag="E")       # exp(transitions)
        ones_cb = small.tile([T, 1], bft, tag="ones_cb")
        ones_cf = small.tile([T, 1], f32, tag="ones_cf")
        st_f = small.tile([T, 1], f32, tag="st_f")
        en_f = small.tile([T, 1], f32, tag="en_f")
        exp_st = small.tile([T, 1], f32, tag="exp_st")
        exp_en = small.tile([T, 1], f32, tag="exp_en")
        nbias = small.tile([T, 1], f32, tag="nbias")
        lnb = small.tile([1, 2048], f32, tag="lnb")
        rplus = small.tile([1, BS], f32, tag="rplus")
        rminus = small.tile([1, BS], f32, tag="rminus")
        scs = small.tile([1, BS], f32, tag="scs")
        acc = small.tile([1, BS], f32, tag="acc")
        acc2 = small.tile([1, BS], f32, tag="acc2")
        out_sb = small.tile([1, BS], f32, tag="out_sb")

        # ---- PSUM ----
        v_A = ppool.tile([T, HALF], f32, tag="vA")
        v_B = ppool.tile([T, HALF], f32, tag="vB")
        g_ps0 = ppool.tile([1, 512], f32, tag="g0")
        g_ps1 = ppool.tile([1, 512], f32, tag="g1")
        sc_ps = ppool.tile([1, NQ * BS], f32, tag="scps")

        # ================= setup =================
        nc.vector.memset(ones_cb, 1.0)
        nc.vector.memset(ones_cf, 1.0)
        nc.vector.memset(nbias, -C_BIAS)
        nc.vector.memset(G[:, 2016:2048], 1.0)        # pad (excluded from reduces)

        em_ap = em_d.ap()
        # first block first (chain seeds), then transitions, then the rest
        nc.sync.dma_start(out=emt[0], in_=em_ap[:, 0:BLK])
        nc.scalar.dma_start(out=E_raw, in_=trf_d.ap())
        nc.scalar.dma_start(out=st_f, in_=stf_d.ap())
        nc.scalar.dma_start(out=en_f, in_=enf_d.ap())
        nc.scalar.activation(E_sb, E_raw, ACTF.Exp)
        nc.scalar.activation(exp_st, st_f, ACTF.Exp)
        nc.scalar.activation(exp_en, en_f, ACTF.Exp)
        nc.scalar.activation(xt[0], emt[0], ACTF.Exp, bias=nbias[:, :])
        for k in range(1, L):
            eng = nc.sync if k % 2 == 0 else nc.scalar
            eng.dma_start(out=emt[k], in_=em_ap[:, k * BLK:(k + 1) * BLK])
            nc.scalar.activation(xt[k], emt[k], ACTF.Exp, bias=nbias[:, :])
        nc.sync.dma_start(out=scp, in_=sc_d.ap())

        # ---- seeds: w = x at t=pL (chain 0 gets exp(st) factor) ----
        nc.vector.tensor_scalar(out=w_A[:, 0:BS], in0=xt[0][:, 0:BS],
                                scalar1=exp_st[:, :], scalar2=None, op0=ALU.mult)
        nc.vector.tensor_copy(w_A[:, BS:HALF], xt[0][:, BS:HALF])
        nc.vector.tensor_copy(w_B, xt[0][:, HALF:BLK])

        # ================= slot loop =================
        # group A: even chains p=0,2..30; group B: odd chains p=1,3..31
        # (chain 31 stops after slot L-1; its final stays in w_B[:,480:512])
        for k in range(1, KMAX + 1):
            for gi, (w, v) in enumerate(((w_A, v_A), (w_B, v_B))):
                if k < L:
                    xa = xt[k][:, 0:HALF] if gi == 0 else xt[k][:, HALF:BLK]
                    cols = HALF
                else:
                    j = k - L
                    if gi == 0:
                        xa = xt[j][:, HALF:BLK]       # odd positions 1..31
                        cols = HALF
                    else:
                        xa = xt[j][:, BS:HALF]        # even positions 2..30
                        cols = HALF - BS
                nc.tensor.matmul(v[:, 0:cols], lhsT=E_sb[:, :], rhs=w[:, 0:cols],
                                 start=True, stop=True)
                nc.vector.tensor_tensor(out=w[:, 0:cols], in0=xa,
                                        in1=v[:, 0:cols], op=ALU.mult)
            if k == KAP:
                nc.vector.tensor_copy(sn_A, w_A)
                nc.vector.tensor_copy(sn_B, w_B)

        # ================= glue =================
        # ratio numerators: <final_p, snap_{p+1}>
        nc.vector.tensor_tensor(out=G[:, 0:512], in0=w_A[:, :],
                                in1=sn_B[:, :], op=ALU.mult)          # p even
        nc.vector.tensor_tensor(out=G[:, 512:992], in0=w_B[:, 0:480],
                                in1=sn_A[:, BS:HALF], op=ALU.mult)    # p odd
        # final Z dot: chain 31 state at t=S-1 times exp(en)
        nc.vector.tensor_scalar(out=G[:, 992:1024], in0=w_B[:, 480:512],
                                scalar1=exp_en[:, :], scalar2=None, op0=ALU.mult)
        # ratio denominators: |snap|^2 for chains 1..31
        nc.vector.tensor_tensor(out=G[:, 1024:1536], in0=sn_B[:, :],
                                in1=sn_B[:, :], op=ALU.mult)          # odd chains
        nc.vector.tensor_tensor(out=G[:, 1536:2016], in0=sn_A[:, BS:HALF],
                                in1=sn_A[:, BS:HALF], op=ALU.mult)    # even chains

        for ci, gp in enumerate((g_ps0, g_ps1, g_ps0, g_ps1)):
            nc.tensor.matmul(gp, lhsT=ones_cb[:, :], rhs=G[:, ci * 512:(ci + 1) * 512],
                             start=True, stop=True)
            nc.scalar.activation(lnb[:, ci * 512:(ci + 1) * 512], gp, ACTF.Ln)

        ln_pos = lnb[:, 0:1024].rearrange("o (c b) -> o b c", b=BS)
        ln_neg = lnb[:, 1024:2016].rearrange("o (c b) -> o b c", b=BS)
        nc.vector.tensor_reduce(rplus, ln_pos, axis=mybir.AxisListType.X, op=ALU.add)
        nc.vector.tensor_reduce(rminus, ln_neg, axis=mybir.AxisListType.X, op=ALU.add)

        # ================= gold-path score =================
        nc.tensor.matmul(sc_ps, lhsT=ones_cf[:, :], rhs=scp[:, :],
                         start=True, stop=True)
        sc3 = sc_ps[0:1, :].rearrange("o (q b) -> o b q", b=BS)
        nc.vector.tensor_reduce(scs, sc3, axis=mybir.AxisListType.X, op=ALU.add)

        # ================= final assembly =================
        nc.vector.tensor_tensor(out=acc, in0=rplus[:, :], in1=rminus[:, :],
                                op=ALU.subtract)
        nc.vector.tensor_scalar(out=acc2, in0=acc[:, :], scalar1=float(S * C_BIAS),
                                scalar2=None, op0=ALU.add)
        nc.vector.tensor_tensor(out=out_sb, in0=acc2[:, :], in1=scs[:, :],
                                op=ALU.subtract)
        nc.sync.dma_start(out=out_d.ap(), in_=out_sb)

    nc.compile()
    return nc


def _host_prep(emissions, tags, transitions, start_transitions, end_transitions):
    """Per-core input maps. Only integer indexing + dtype/layout prep."""
    em_all = np.asarray(emissions, np.float32)
    tg_all = np.asarray(tags).astype(np.int64)
    trf = np.ascontiguousarray(np.asarray(transitions, np.float32))
    stf = np.asarray(start_transitions, np.float32).reshape(T, 1)
    enf = np.asarray(end_transitions, np.float32).reshape(T, 1)
    in_maps = []
    for c in range(NCORES):
        emc = em_all[c * BS:(c + 1) * BS]               # [BS, S, T]
        tg = tg_all[c * BS:(c + 1) * BS]                # [BS, S]
        # recurrence layout: col = k*1024 + parity*512 + (p//2)*32 + b
        em_slot = (emc.transpose(2, 1, 0)               # [tag, t, b]
                   .reshape(T, P, L, BS)                # t = p*L + k
                   .reshape(T, P // 2, 2, L, BS)        # p = ph*2 + parity
                   .transpose(0, 3, 2, 1, 4)            # [tag, k, par, ph, b]
                   .reshape(T, S * BS)).astype(bf16)
        # score pack: vals[b, q*128 + r] -> scp[r, q*32 + b]
        emit_sc = np.take_along_axis(emc, tg[..., None], axis=2)[..., 0]
        vals = np.zeros((BS, NQ * T), np.float32)
        vals[:, :S] = emit_sc
        vals[:, S:S + S - 1] = trf[tg[:, :-1], tg[:, 1:]]
        vals[:, S + S - 1] = stf[tg[:, 0], 0]
        vals[:, S + S] = enf[tg[:, -1], 0]
        scp = (vals.reshape(BS, NQ, T).transpose(2, 1, 0)
               .reshape(T, NQ * BS))
        in_maps.append({
            "em": np.ascontiguousarray(em_slot),
            "sc": np.ascontiguousarray(scp),
            "trf": trf, "stf": stf, "enf": enf,
        })
    return in_maps


def _numpy_fallback(emissions, tags, mask, transitions, start_transitions,
                    end_transitions):
    em = np.asarray(emissions, np.float32)
    tg = np.asarray(tags).astype(np.int64)
    mk = np.asarray(mask).astype(np.float32)
    tr = np.asarray(transitions, np.float32)
    st = np.asarray(start_transitions, np.float32)
    en = np.asarray(end_transitions, np.float32)
    Bn, Sn, Tn = em.shape
    score = st[tg[:, 0]]
    emit = np.take_along_axis(em, tg[..., None], axis=2)[..., 0]
    score = score + (emit * mk).sum(1)
    score = score + (tr[tg[:, :-1], tg[:, 1:]] * mk[:, 1:]).sum(1)
    last = mk.astype(np.int64).sum(1) - 1
    score = score + en[np.take_along_axis(tg, last[:, None], 1)[:, 0]]
    fv = st[None, :] + em[:, 0]
    for t in range(1, Sn):
        m = fv.max(1, keepdims=True)
        fv = np.log(np.exp(fv - m) @ np.exp(tr)) + m + em[:, t]
    m = fv.max(1, keepdims=True)
    part = np.log((np.exp(fv - m) * np.exp(en)[None, :]).sum(1)) + m[:, 0]
    return -(score - part)


def kernel(emissions, tags, mask, transitions, start_transitions,
           end_transitions):
    em_arr = np.asarray(emissions)
    mask_arr = np.asarray(mask)
    tg_arr = np.asarray(tags).astype(np.int64)
    off_spec = (
        em_arr.shape != (B, S, T)
        or not mask_arr.all()
        or tg_arr.min() < 0 or tg_arr.max() >= T
    )
    if off_spec:
        return _numpy_fallback(emissions, tags, mask, transitions,
                               start_transitions, end_transitions).astype(np.float32)

    from concourse import bass_utils

    if "nc" not in _CACHED:
        _CACHED["nc"] = _build_bass()
    nc = _CACHED["nc"]

    in_maps = _host_prep(emissions, tags, transitions, start_transitions,
                         end_transitions)
    res = bass_utils.run_bass_kernel_spmd(nc, in_maps, core_ids=list(range(NCORES)))
    out = np.concatenate([np.asarray(res.results[c]["out"]).reshape(BS)
                          for c in range(NCORES)])
    return out.astype(np.float32)
